# revision 1
# baseline (speedup 1.0000x reference)
"""Trainium2 Bass kernel for nn_Decoder (3-layer GNN message-passing decoder).

Sharding: node axis split across 8 cores (2500 nodes/core), weights replicated.
All on-device tensors live in [feature=128 partitions, free] layout; the host
pre-transposes edge/node features (and casts to bf16) so the device never
transposes anything, and transposes the [C, n] output back at the end.

Per-core, per-layer structure (T=500-node tiles, K=32 edge slots/node):
  S        = W1a@h + W1b@nf                       (per-node part of mm1, PE)
  m1[k]    = gelu(W1e@efT[k] + S + b1)            (PE + DVE bcast-add + ACT)
  m2[k]    = gelu(W2@m1[k] + b2)                  (PE + ACT)
  acc      = h + sum_k (W3/30)@m2[k]              (PSUM-accumulated over k, PE;
                                                   h preloaded via identity matmul)
  h        = LN(acc + K*b3/30)                    (stats via ones-matmul over
                                                   partitions; 1/sqrt via exp(-0.5*ln))
  h        = LN(h + do@gelu(di@h)) * mask
"""

import sys
from contextlib import ExitStack

for _p in ("/opt/trn_rl_repo", "/root/.axon_site/_ro/trn_rl_repo"):
    if _p not in sys.path:
        sys.path.append(_p)

import numpy as np
import ml_dtypes

import concourse.bass as bass
import concourse.tile as tile
from concourse import bacc, mybir
from concourse.bass_utils import run_bass_kernel_spmd
from concourse.masks import make_identity

N, K, C, H, L = 20000, 32, 128, 128, 3
NCORES = 8
NPER = N // NCORES          # 2500 nodes per core
T = 500                     # node tile (NPER divisible)
NT = NPER // T              # 5 tiles
KGRP = 2                    # k-slices per psum group (2*512 fp32 = 2 PSUM banks)
SCALE, EPS = 30.0, 1e-5

BF = mybir.dt.bfloat16
F32 = mybir.dt.float32
AF = mybir.ActivationFunctionType
OP = mybir.AluOpType


def _emit(ctx, tc, io, nper, tsz):
    nc = tc.nc
    nt = nper // tsz
    ngrp = K // KGRP

    consts = ctx.enter_context(tc.tile_pool(name="consts", bufs=1))
    efpool = ctx.enter_context(tc.tile_pool(name="ef", bufs=2))
    spool = ctx.enter_context(tc.tile_pool(name="sp", bufs=6))
    mdpool = ctx.enter_context(tc.tile_pool(name="md", bufs=5))
    tmppool = ctx.enter_context(tc.tile_pool(name="tmp", bufs=3))
    stgpool = ctx.enter_context(tc.tile_pool(name="stg", bufs=2))
    psmain = ctx.enter_context(tc.tile_pool(name="psmain", bufs=3, space="PSUM"))
    psacc = ctx.enter_context(tc.tile_pool(name="psacc", bufs=1, space="PSUM"))
    psmisc = ctx.enter_context(tc.tile_pool(name="psmisc", bufs=1, space="PSUM"))

    # ---- persistent SBUF state ----
    nfh = consts.tile([C, nper], BF, tag="nfh")            # node features == h0
    mask_rep = consts.tile([C, nper], BF, tag="maskr")
    h_bufs = [consts.tile([C, nper], BF, tag=f"hbuf{i}", name=f"hbuf{i}")
              for i in range(2)]
    h1_sb = consts.tile([C, nper], BF, tag="h1")
    x2t = consts.tile([C, nper], BF, tag="x2t")            # pre-LN x for stats/apply
    # half-tile m1/m2 staging (gelu1 out, overwritten in place by gelu2 out)
    m12 = [consts.tile([C, K // 2, tsz], BF, tag=f"m12{i}", name=f"m12{i}")
           for i in range(2)]
    mean_sb = consts.tile([C, nper], F32, tag="mean")
    es2_sb = consts.tile([C, nper], F32, tag="es2")
    u_sb = consts.tile([C, nper], F32, tag="u")
    inv_sb = consts.tile([C, nper], BF, tag="inv")

    wts = {}
    for nm in ("w1aT", "w1bT", "w1eT", "w2T", "w3sT", "diwT", "dowT"):
        wt = consts.tile([C, L, H], BF, tag=nm, name=nm)
        for l in range(L):
            nc.sync.dma_start(out=wt[:, l, :], in_=io[nm][l, :, :])
        wts[nm] = wt
    bvec = consts.tile([C, 15], F32, tag="bvec")
    nc.sync.dma_start(out=bvec[:, :], in_=io["bvec"][:, :])
    lnvec = consts.tile([C, 12], F32, tag="lnvec")
    nc.sync.dma_start(out=lnvec[:, :], in_=io["lnvec"][:, :])

    ident = consts.tile([C, C], BF, tag="ident")
    make_identity(nc, ident[:, :])
    ones_t = consts.tile([C, C], BF, tag="ones")
    nc.vector.memset(ones_t[:, :], 1.0)
    eps_sb = consts.tile([C, 1], F32, tag="eps")
    nc.vector.memset(eps_sb[:, :], EPS)

    nc.sync.dma_start(out=nfh[:, :], in_=io["nfT"][:, :])
    _m = io["maskT"]
    _mb = bass.AP(tensor=_m.tensor, offset=_m.offset, ap=[[0, C], _m.ap[1]])
    nc.sync.dma_start(out=mask_rep[:, :], in_=_mb)

    s_tiles = {}

    def emit_S(l, t, h_src):
        sl_ = slice(t * tsz, (t + 1) * tsz)
        s_ps = psmisc.tile([C, 512], F32, tag="psS", name="s_ps")
        nc.tensor.matmul(s_ps[:, 0:tsz], wts["w1aT"][:, l, :], h_src[:, sl_],
                         start=True, stop=False)
        nc.tensor.matmul(s_ps[:, 0:tsz], wts["w1bT"][:, l, :], nfh[:, sl_],
                         start=False, stop=True)
        s_sb = spool.tile([C, tsz], BF, tag="ssb", name="s_sb")
        nc.vector.tensor_copy(out=s_sb[:, :], in_=s_ps[:, 0:tsz])
        s_tiles[(l, t)] = s_sb

    def bcol(base, l):
        return bvec[:, base + l:base + l + 1]

    def lncol(base, l):
        return lnvec[:, base + l:base + l + 1]

    for t in range(nt):
        emit_S(0, t, nfh)

    for l in range(L):
        h_cur = nfh if l == 0 else h_bufs[(l + 1) % 2]
        w1a = wts["w1aT"][:, l, :]
        w1b = wts["w1bT"][:, l, :]
        w1e = wts["w1eT"][:, l, :]
        w2 = wts["w2T"][:, l, :]
        w3s = wts["w3sT"][:, l, :]
        diw = wts["diwT"][:, l, :]
        dow = wts["dowT"][:, l, :]

        # ======== edge phase, per node tile ========
        for t in range(nt):
            n0 = t * tsz
            sl = slice(n0, n0 + tsz)
            ef_sb = efpool.tile([C, K, tsz], BF, tag="ef")
            for q in range(4):
                nc.sync.dma_start(out=ef_sb[:, q * 8:(q + 1) * 8, :],
                                  in_=io["efT"][:, q * 8:(q + 1) * 8, sl])

            s_sb = s_tiles.pop((l, t))
            s_ap = s_sb[:, :]
            s_bcast = bass.AP(tensor=s_ap.tensor, offset=s_ap.offset,
                              ap=[s_ap.ap[0], [0, KGRP], s_ap.ap[1]])

            # phase-batched halves: A=mm1e+addS+gelu1, B=mm2+gelu2 (in place),
            # C=dense k-sum matmul tail.  Order A0 B0 A1 C0 B1 C1 keeps the
            # ACT stream free of head-of-line stalls while the C tails give
            # the PE long dense bursts (HAM warm-up).
            KH = K // 2
            GH = KH // 2  # psum groups per half

            def phase_A(h):
                for gq in range(GH // 2):
                    stg = stgpool.tile([C, 4, 512], F32, tag="stg", name="stg")
                    for g2 in range(2):
                        g = gq * 2 + g2
                        pa = psmain.tile([C, 2, 512], F32, tag="pm", name="pa")
                        for j in range(2):
                            k = h * KH + g * 2 + j
                            nc.tensor.matmul(pa[:, j, 0:tsz], w1e, ef_sb[:, k, :],
                                             start=True, stop=True)
                        nc.vector.tensor_add(stg[:, g2 * 2:(g2 + 1) * 2, 0:tsz],
                                             pa[:, :, 0:tsz], s_bcast)
                    nc.scalar.activation(out=m12[h][:, gq * 4:(gq + 1) * 4, 0:tsz],
                                         in_=stg[:, :, 0:tsz],
                                         func=AF.Gelu, bias=bcol(0, l))

            def phase_B(h):
                for g in range(GH):
                    pb = psmain.tile([C, 2, 512], F32, tag="pm", name="pb")
                    for i in range(2):
                        nc.tensor.matmul(pb[:, i, 0:tsz], w2,
                                         m12[h][:, g * 2 + i, 0:tsz],
                                         start=True, stop=True)
                    nc.scalar.activation(out=m12[h][:, g * 2:(g + 1) * 2, 0:tsz],
                                         in_=pb[:, :, 0:tsz],
                                         func=AF.Gelu, bias=bcol(3, l))

            def phase_C(h, acc):
                for kk in range(KH):
                    nc.tensor.matmul(acc[:, 0:tsz], w3s, m12[h][:, kk, 0:tsz],
                                     start=False, stop=(h == 1 and kk == KH - 1))

            phase_A(0)
            phase_B(0)
            phase_A(1)
            acc_ps = psacc.tile([C, 512], F32, tag="acc", name="acc_ps")
            nc.tensor.matmul(acc_ps[:, 0:tsz], ident[:, :], h_cur[:, sl],
                             start=True, stop=False)
            phase_C(0, acc_ps)
            phase_B(1)
            phase_C(1, acc_ps)
            # x = acc + K*b3/30 -> bf16 ; sq = x*x ; partition sums via ones-matmul
            nc.vector.tensor_scalar(x2t[:, sl], acc_ps[:, 0:tsz],
                                    bcol(6, l), None, OP.add)
            sq = tmppool.tile([C, tsz], BF, tag="sq", name="sq", bufs=6)
            nc.vector.tensor_mul(sq[:, :], x2t[:, sl], x2t[:, sl])
            st1 = psmisc.tile([C, 512], F32, tag="psS", name="st1")
            nc.tensor.matmul(st1[:, 0:tsz], ones_t[:, :], x2t[:, sl],
                             start=True, stop=True)
            nc.vector.tensor_scalar(mean_sb[:, sl], st1[:, 0:tsz],
                                    1.0 / C, None, OP.mult)
            st2 = psmisc.tile([C, 512], F32, tag="psS", name="st2")
            nc.tensor.matmul(st2[:, 0:tsz], ones_t[:, :], sq[:, :],
                             start=True, stop=True)
            nc.vector.tensor_scalar(es2_sb[:, sl], st2[:, 0:tsz],
                                    1.0 / C, None, OP.mult)
            nc.vector.tensor_mul(u_sb[:, sl], mean_sb[:, sl], mean_sb[:, sl])
            nc.vector.tensor_sub(u_sb[:, sl], es2_sb[:, sl], u_sb[:, sl])

        # ======== node phase (per layer), phase-batched ========
        def make_inv():
            # inv = exp(-0.5 * ln(var + eps)); var precomputed into u_sb
            nc.scalar.activation(out=u_sb[:, :], in_=u_sb[:, :], func=AF.Ln,
                                 bias=eps_sb[:, :])
            nc.scalar.activation(out=inv_sb[:, :], in_=u_sb[:, :], func=AF.Exp,
                                 scale=-0.5)

        make_inv()  # LN1
        # pass 1: LN1 apply for all tiles
        for t in range(nt):
            sl = slice(t * tsz, (t + 1) * tsz)
            tmp = tmppool.tile([C, tsz], BF, tag="tmp")
            nc.vector.tensor_sub(tmp[:, :], x2t[:, sl], mean_sb[:, sl])
            nc.vector.tensor_mul(tmp[:, :], tmp[:, :], inv_sb[:, sl])
            nc.vector.tensor_scalar(h1_sb[:, sl], tmp[:, :],
                                    lncol(0, l), lncol(3, l), OP.mult, OP.add)
        # pass 2: di matmul + gelu for all tiles
        mds = []
        for t in range(nt):
            sl = slice(t * tsz, (t + 1) * tsz)
            dpa = psmisc.tile([C, 512], F32, tag="psS", name="dpa")
            nc.tensor.matmul(dpa[:, 0:tsz], diw, h1_sb[:, sl], start=True, stop=True)
            md = mdpool.tile([C, tsz], BF, tag="md", name="md")
            nc.scalar.activation(out=md[:, :], in_=dpa[:, 0:tsz], func=AF.Gelu,
                                 bias=bcol(9, l))
            mds.append(md)
        # pass 3 (sub-phase batched): do-matmuls, then x2/sq, then stats
        for t in range(nt):
            sl = slice(t * tsz, (t + 1) * tsz)
            dpb = psmisc.tile([C, 512], F32, tag="psS", name="dpb")
            nc.tensor.matmul(dpb[:, 0:tsz], ident[:, :], h1_sb[:, sl],
                             start=True, stop=False)
            nc.tensor.matmul(dpb[:, 0:tsz], dow, mds[t][:, :], start=False, stop=True)
            nc.vector.tensor_scalar(x2t[:, sl], dpb[:, 0:tsz],
                                    bcol(12, l), None, OP.add)
        sq2s = []
        for t in range(nt):
            sl = slice(t * tsz, (t + 1) * tsz)
            sq2 = tmppool.tile([C, tsz], BF, tag="sq", name="sq2", bufs=6)
            nc.vector.tensor_mul(sq2[:, :], x2t[:, sl], x2t[:, sl])
            sq2s.append(sq2)
        for t in range(nt):
            sl = slice(t * tsz, (t + 1) * tsz)
            dpc = psmisc.tile([C, 512], F32, tag="psS", name="dpc")
            nc.tensor.matmul(dpc[:, 0:tsz], ones_t[:, :], x2t[:, sl],
                             start=True, stop=True)
            nc.vector.tensor_scalar(mean_sb[:, sl], dpc[:, 0:tsz],
                                    1.0 / C, None, OP.mult)
        for t in range(nt):
            sl = slice(t * tsz, (t + 1) * tsz)
            dpd = psmisc.tile([C, 512], F32, tag="psS", name="dpd")
            nc.tensor.matmul(dpd[:, 0:tsz], ones_t[:, :], sq2s[t][:, :],
                             start=True, stop=True)
            nc.vector.tensor_scalar(es2_sb[:, sl], dpd[:, 0:tsz],
                                    1.0 / C, None, OP.mult)
        for t in range(nt):
            sl = slice(t * tsz, (t + 1) * tsz)
            nc.vector.tensor_mul(u_sb[:, sl], mean_sb[:, sl], mean_sb[:, sl])
            nc.vector.tensor_sub(u_sb[:, sl], es2_sb[:, sl], u_sb[:, sl])

        make_inv()  # LN2
        for t in range(nt):
            sl = slice(t * tsz, (t + 1) * tsz)
            tmp = tmppool.tile([C, tsz], BF, tag="tmp")
            nc.vector.tensor_sub(tmp[:, :], x2t[:, sl], mean_sb[:, sl])
            nc.vector.tensor_mul(tmp[:, :], tmp[:, :], inv_sb[:, sl])
            if l < L - 1:
                q = tmppool.tile([C, tsz], BF, tag="q")
                nc.vector.tensor_scalar(q[:, :], tmp[:, :],
                                        lncol(6, l), lncol(9, l), OP.mult, OP.add)
                nc.vector.tensor_mul(h_bufs[l % 2][:, sl], q[:, :], mask_rep[:, sl])
                emit_S(l + 1, t, h_bufs[l % 2])
            else:
                q = tmppool.tile([C, tsz], BF, tag="q")
                nc.vector.tensor_scalar(q[:, :], tmp[:, :],
                                        lncol(6, l), lncol(9, l), OP.mult, OP.add)
                nc.vector.tensor_mul(mean_sb[:, sl], q[:, :], mask_rep[:, sl])
                nc.sync.dma_start(out=io["out_hT"][:, sl], in_=mean_sb[:, sl])


def build_nc(nper=NPER, tsz=T):
    nc = bacc.Bacc("TRN2", target_bir_lowering=False, debug=False,
                   enable_asserts=False)
    io = {
        "efT": nc.dram_tensor("efT", [C, K, nper], BF, kind="ExternalInput").ap(),
        "nfT": nc.dram_tensor("nfT", [C, nper], BF, kind="ExternalInput").ap(),
        "maskT": nc.dram_tensor("maskT", [1, nper], BF, kind="ExternalInput").ap(),
        "bvec": nc.dram_tensor("bvec", [C, 15], F32, kind="ExternalInput").ap(),
        "lnvec": nc.dram_tensor("lnvec", [C, 12], F32, kind="ExternalInput").ap(),
        "out_hT": nc.dram_tensor("out_hT", [C, nper], F32, kind="ExternalOutput").ap(),
    }
    for nm in ("w1aT", "w1bT", "w1eT", "w2T", "w3sT", "diwT", "dowT"):
        io[nm] = nc.dram_tensor(nm, [L, C, H], BF, kind="ExternalInput").ap()
    with tile.TileContext(nc) as tc:
        with ExitStack() as ctx:
            _emit(ctx, tc, io, nper, tsz)
    nc.compile()
    return nc


def host_prep(inputs, nper=NPER, ncores=NCORES):
    """Shard + lay out inputs for the device. Returns list of per-core in_maps."""
    bf = ml_dtypes.bfloat16
    nf = np.asarray(inputs["node_features"], np.float32)
    ef = np.asarray(inputs["edge_features"], np.float32)
    mask = np.asarray(inputs["mask"], np.float32)
    w1 = np.asarray(inputs["w1"], np.float32)
    w2 = np.asarray(inputs["w2"], np.float32)
    w3 = np.asarray(inputs["w3"], np.float32)
    di_w = np.asarray(inputs["di_w"], np.float32)
    do_w = np.asarray(inputs["do_w"], np.float32)

    def tr(w):  # (L, A, B) -> (L, B, A) contiguous bf16
        return np.ascontiguousarray(w.transpose(0, 2, 1)).astype(bf)

    shared = {
        "w1aT": tr(w1[:, :, 0:C]),
        "w1bT": tr(w1[:, :, C:2 * C]),
        "w1eT": tr(w1[:, :, 3 * C:4 * C]),
        "w2T": tr(w2),
        "w3sT": tr(w3 / SCALE),
        "diwT": tr(di_w),
        "dowT": tr(do_w),
    }
    bvec = np.zeros((C, 15), np.float32)
    lnvec = np.zeros((C, 12), np.float32)
    for l in range(L):
        bvec[:, 0 + l] = np.asarray(inputs["b1"][l], np.float32)
        bvec[:, 3 + l] = np.asarray(inputs["b2"][l], np.float32)
        bvec[:, 6 + l] = np.asarray(inputs["b3"][l], np.float32) * K / SCALE
        bvec[:, 9 + l] = np.asarray(inputs["di_b"][l], np.float32)
        bvec[:, 12 + l] = np.asarray(inputs["do_b"][l], np.float32)
        lnvec[:, 0 + l] = np.asarray(inputs["n1_s"][l], np.float32)
        lnvec[:, 3 + l] = np.asarray(inputs["n1_b"][l], np.float32)
        lnvec[:, 6 + l] = np.asarray(inputs["n2_s"][l], np.float32)
        lnvec[:, 9 + l] = np.asarray(inputs["n2_b"][l], np.float32)
    shared["bvec"] = bvec
    shared["lnvec"] = lnvec

    in_maps = []
    for c in range(ncores):
        sl = slice(c * nper, (c + 1) * nper)
        efc = ef[sl].astype(bf)                              # (nper, K, C)
        in_maps.append(dict(
            efT=np.ascontiguousarray(efc.transpose(2, 1, 0)),  # (C, K, nper)
            nfT=np.ascontiguousarray(nf[sl].T).astype(bf),
            maskT=mask[sl].reshape(1, nper).astype(bf),
            **shared,
        ))
    return in_maps


_NC_CACHE = {}


def kernel(**inputs):
    in_maps = host_prep(inputs)
    if "nc" not in _NC_CACHE:
        _NC_CACHE["nc"] = build_nc()
    nc = _NC_CACHE["nc"]
    res = run_bass_kernel_spmd(nc, in_maps, core_ids=list(range(NCORES)))
    out = np.concatenate([np.asarray(res.results[c]["out_hT"]).T
                          for c in range(NCORES)], axis=0)
    return np.ascontiguousarray(out.astype(np.float32))



# revision 8
# speedup vs baseline: 1.2072x; 1.2072x over previous
"""Trainium2 Bass kernel for nn_Decoder (3-layer GNN message-passing decoder).

Sharding: node axis split across 8 cores (2500 nodes/core), weights replicated.
Device layout: [feature=128 partitions, free]; host pre-transposes, output
transposed back.

Engine split (per 500-node tile, K=32 edge slots):
  PE : z1[k] = W1e'@ef[k]; z2[k] = W2'@m1[k]; acc += W3s@m2[k];
       S' = W1a'@h + W1b'@nf; LN stats via (1/C)-matmul; di/do matmuls.
  DVE: m1[k] = gelu~(z1[k] + S') in ONE custom 8-op instruction
       (smoothstep gelu: s=clamp01(p); out=(p-.5)*s^2*(1.5-s); input
       pre-scaled by LAM in the weights, output scale folded into W2'),
       plus var/LN-tail custom ops.
  ACT: m2[k] = gelu(z2[k]+b2) exact; copies with bias; Ln/Exp for 1/sqrt(var).
  Pool(GPSIMD): LN elementwise (x-mean)*inv, squares.
h and pre-LN x kept in fp32 (f32r moving matmuls) for accuracy headroom.
"""

import sys
from contextlib import ExitStack

for _p in ("/opt/trn_rl_repo", "/root/.axon_site/_ro/trn_rl_repo"):
    if _p not in sys.path:
        sys.path.append(_p)

import numpy as np
import ml_dtypes

import concourse.bass as bass
import concourse.tile as tile
from concourse import bacc, mybir
from concourse.bass_utils import run_bass_kernel_spmd
from concourse.masks import make_identity

N, K, C, H, L = 20000, 32, 128, 128, 3
NCORES = 8
NPER = N // NCORES          # 2500 nodes per core
T = 500                     # node tile
SCALE, EPS = 30.0, 1e-5

# smoothstep-gelu params: gelu(z) ~= MU * (LAM z) * s^2 (C2F - s),
# s = clamp01(LAM z + C1F)
LAM, C1F, C2F = 0.221791, 0.5, 1.5
MU = 1.0 / (LAM * (C2F - 1.0))

BF = mybir.dt.bfloat16
F32 = mybir.dt.float32
F32R = mybir.dt.float32r
AF = mybir.ActivationFunctionType
OP = mybir.AluOpType

# ---------------- custom DVE ops (registered at import) ----------------


def _register_dve_ops():
    from concourse import dve_ops as dvo
    from concourse.dve_spec import (Spec, Src0, Src1, C0, C1, C2, One,
                                    relu, minn, lower, _has_src1)
    from concourse.dve_uop import DveOpSpec

    def _ref_gelu1(in0, in1, s0, s1, imm2):
        p = in0.astype(np.float32) + in1
        s = np.clip(p, 0.0, 1.0)
        return (s * s * (imm2 - s)) * (p + s1)

    def _ref_gelu2(in0, in1, s0, s1, imm2):
        s0 = np.asarray(s0, np.float32).reshape(-1, 1)
        s1 = np.asarray(s1, np.float32).reshape(-1, 1)
        sig0 = in0.astype(np.float32) + s1
        s = np.clip(sig0, 0.0, 1.0)
        return (s * s * (imm2 - s)) * (in0 + s0)

    def _ref_var2(in0, in1, s0, s1, imm2):
        return in0.astype(np.float32) - in1.astype(np.float32) * in1

    def g1_spec():
        p = Src0 + Src1
        s = minn(relu(p), One)
        return Spec(body=(s * s) * (C2 - s) * (p + C1), reference=_ref_gelu1)

    def g2_spec():
        sig0 = Src0 + C1
        s = minn(relu(sig0), One)
        return Spec(body=(s * s) * (C2 - s) * (Src0 + C0), reference=_ref_gelu2)

    def var2_spec():
        return Spec(body=Src0 - Src1 * Src1, reference=_ref_var2)

    out = []
    for name, mk in (("GELU1_SMST_ANT", g1_spec), ("GELU2_SMST_ANT", g2_spec),
                     ("VAR2_MEANS_ANT", var2_spec)):
        existing = next((o for o in dvo.OPS if o.name == name), None)
        if existing is not None:
            out.append(existing)
            continue
        spec = mk()
        row = max(dvo._SUB_OPCODE_FOR_NAME.values()) + 1
        assert row < 0x20
        dvo._SUB_OPCODE_FOR_NAME[name] = row
        shas = {}
        for ver in ("v3", "v4"):
            tmp = DveOpSpec(name=name, opcode=row, uops=lower(spec, ver=ver),
                            rd1_en=_has_src1(spec))
            shas[ver] = tmp.sha(ver)
        op = dvo.DveOp(name, spec, subdim=False, uops_sha=shas)
        dvo.OPS.append(op)
        dvo.CUSTOM_DVE_SPECS[name] = spec
        out.append(op)
    return out


GELU1_OP, GELU2_OP, VAR2_OP = _register_dve_ops()
from concourse.dve_ops import AFFINE_MUL_REDUCE  # noqa: E402


def _emit(ctx, tc, io, nper, tsz):
    nc = tc.nc
    nt = nper // tsz

    consts = ctx.enter_context(tc.tile_pool(name="consts", bufs=1))
    efpool = ctx.enter_context(tc.tile_pool(name="ef", bufs=2))
    spool = ctx.enter_context(tc.tile_pool(name="sp", bufs=6))
    m1pool = ctx.enter_context(tc.tile_pool(name="m1", bufs=3))
    m2pool = ctx.enter_context(tc.tile_pool(name="m2", bufs=3))
    mdpool = ctx.enter_context(tc.tile_pool(name="md", bufs=5))
    sqpool = ctx.enter_context(tc.tile_pool(name="sq", bufs=2))
    psz1 = ctx.enter_context(tc.tile_pool(name="psz1", bufs=2, space="PSUM"))
    psz2 = ctx.enter_context(tc.tile_pool(name="psz2", bufs=2, space="PSUM"))
    psacc = ctx.enter_context(tc.tile_pool(name="psacc", bufs=2, space="PSUM"))
    psst = ctx.enter_context(tc.tile_pool(name="psst", bufs=1, space="PSUM"))

    # ---- persistent SBUF state ----
    nfh = consts.tile([C, nper], BF, tag="nfh")
    mask_rep = consts.tile([C, nper], BF, tag="maskr")
    h_bufs = [consts.tile([C, nper], F32, tag=f"hbuf{i}", name=f"hbuf{i}")
              for i in range(2)]
    x2t = consts.tile([C, nper], F32, tag="x2t")
    xn = consts.tile([C, nper], F32, tag="xn")
    xnbf = consts.tile([C, nper], BF, tag="xnbf")
    hsh = consts.tile([C, nper], BF, tag="hsh")
    mean_sb = consts.tile([C, nper], F32, tag="mean")
    var_sb = consts.tile([C, nper], F32, tag="var")
    inv_sb = consts.tile([C, nper], F32, tag="inv")

    wts = {}
    for nm in ("w1aT", "w1bT", "w1eT", "w2T", "w3sT", "diwT", "dowT"):
        wt = consts.tile([C, L, H], BF, tag=nm, name=nm)
        for l in range(L):
            nc.sync.dma_start(out=wt[:, l, :], in_=io[nm][l, :, :])
        wts[nm] = wt
    bvec = consts.tile([C, 15], F32, tag="bvec")
    nc.sync.dma_start(out=bvec[:, :], in_=io["bvec"][:, :])
    lnvec = consts.tile([C, 12], F32, tag="lnvec")
    nc.sync.dma_start(out=lnvec[:, :], in_=io["lnvec"][:, :])

    oneC = consts.tile([C, C], BF, tag="oneC")
    nc.vector.memset(oneC[:, :], 1.0 / C)
    eps_sb = consts.tile([C, 1], F32, tag="eps")
    nc.vector.memset(eps_sb[:, :], EPS)

    nc.sync.dma_start(out=nfh[:, :], in_=io["nfT"][:, :])
    _m = io["maskT"]
    _mb = bass.AP(tensor=_m.tensor, offset=_m.offset, ap=[[0, C], _m.ap[1]])
    nc.sync.dma_start(out=mask_rep[:, :], in_=_mb)

    s_tiles = {}

    def bcol(base, l):
        return bvec[:, base + l:base + l + 1]

    def lncol(base, l):
        return lnvec[:, base + l:base + l + 1]

    def emit_S(l, t, h_src):
        sl = slice(t * tsz, (t + 1) * tsz)
        sp = psst.tile([C, 2, 512], F32, tag="st", name="sps")
        nc.tensor.matmul(sp[:, 0, 0:tsz], wts["w1aT"][:, l, :], h_src[:, sl],
                         start=True, stop=False)
        nc.tensor.matmul(sp[:, 0, 0:tsz], wts["w1bT"][:, l, :], nfh[:, sl],
                         start=False, stop=True)
        s_sb = spool.tile([C, tsz], F32, tag="ssb", name="s_sb")
        nc.scalar.activation(out=s_sb[:, :], in_=sp[:, 0, 0:tsz],
                             func=AF.Identity, bias=bcol(0, l))
        s_tiles[(l, t)] = s_sb

    def stats_of(src_sl, sq_src):
        """LN stats: mean & E[x^2] via (1/C)-matmuls on bf16 shadows."""
        xbf = sqpool.tile([C, tsz], BF, tag="xbf", name="xbf")
        nc.gpsimd.tensor_copy(xbf[:, :], sq_src)
        sq = sqpool.tile([C, tsz], BF, tag="sq", name="sq")
        nc.gpsimd.tensor_mul(sq[:, :], xbf[:, :], xbf[:, :])
        st = psst.tile([C, 2, 512], F32, tag="st", name="st")
        nc.tensor.matmul(st[:, 0, 0:tsz], oneC[:, :], xbf[:, :],
                         start=True, stop=True)
        nc.tensor.matmul(st[:, 1, 0:tsz], oneC[:, :], sq[:, :],
                         start=True, stop=True)
        nc.scalar.activation(out=mean_sb[:, src_sl], in_=st[:, 0, 0:tsz],
                             func=AF.Copy)
        nc.vector._custom_dve(VAR2_OP, out=var_sb[:, src_sl],
                              in0=st[:, 1, 0:tsz], in1=mean_sb[:, src_sl])

    for t in range(nt):
        emit_S(0, t, nfh)

    for l in range(L):
        h_cur = h_bufs[(l + 1) % 2]      # f32 master (unused at l==0)
        h_cur_sh = nfh if l == 0 else hsh  # bf16 shadow for S-matmuls
        w1e = wts["w1eT"][:, l, :]
        w2 = wts["w2T"][:, l, :]
        w3s = wts["w3sT"][:, l, :]
        diw = wts["diwT"][:, l, :]
        dow = wts["dowT"][:, l, :]

        # ======== edge phase, per node tile ========
        # Tile epilogue (x2t copy + LN stats) is DEFERRED into the next
        # tile's k-loop so PE's acc-preload/A-matmuls aren't head-of-line
        # blocked behind ACT/Pool-gated stats work.
        pending = None

        def flush_pending():
            nonlocal pending
            if pending is None:
                return
            p_sl, p_acc = pending
            pending = None
            nc.scalar.activation(out=x2t[:, p_sl], in_=p_acc[:, 0:tsz],
                                 func=AF.Identity, bias=bcol(6, l))
            h_res = nfh if l == 0 else h_cur
            nc.gpsimd.tensor_add(x2t[:, p_sl], x2t[:, p_sl], h_res[:, p_sl])
            stats_of(p_sl, x2t[:, p_sl])

        for t in range(nt):
            sl = slice(t * tsz, (t + 1) * tsz)
            ef_sb = efpool.tile([C, K, tsz], BF, tag="ef")
            for q in range(4):
                nc.sync.dma_start(out=ef_sb[:, q * 8:(q + 1) * 8, :],
                                  in_=io["efT"][:, q * 8:(q + 1) * 8, sl])
            s_sb = s_tiles.pop((l, t))

            acc = psacc.tile([C, 512], F32, tag="acc", name="acc")

            z1t, z2t = {}, {}
            for i in range(K + 2):
                if i == 2:
                    flush_pending()
                if i < K:
                    z1 = psz1.tile([C, 512], F32, tag="z1", name="z1")
                    nc.tensor.matmul(z1[:, 0:tsz], w1e, ef_sb[:, i, :],
                                     start=True, stop=True)
                    z1t[i] = z1
                k2 = i - 2
                if k2 >= 0:
                    m2 = m2pool.tile([C, tsz], BF, tag="m2", name="m2")
                    nc.scalar.activation(out=m2[:, :], in_=z2t.pop(k2)[:, 0:tsz],
                                         func=AF.Gelu, bias=bcol(3, l))
                    nc.tensor.matmul(acc[:, 0:tsz], w3s, m2[:, :],
                                     start=(k2 == 0), stop=(k2 == K - 1))
                k1 = i - 1
                if 0 <= k1 < K:
                    m1 = m1pool.tile([C, tsz], BF, tag="m1", name="m1")
                    nc.vector._custom_dve(GELU1_OP, out=m1[:, :],
                                          in0=z1t.pop(k1)[:, 0:tsz],
                                          in1=s_sb[:, :], s0=0.0, s1=-C1F,
                                          imm2=C2F)
                    z2 = psz2.tile([C, 512], F32, tag="z2", name="z2")
                    nc.tensor.matmul(z2[:, 0:tsz], w2, m1[:, :],
                                     start=True, stop=True)
                    z2t[k1] = z2

            pending = (sl, acc)
        flush_pending()

        # ======== node phase ========
        # inv = exp(-0.5*ln(var+eps))
        nc.scalar.activation(out=var_sb[:, :], in_=var_sb[:, :], func=AF.Ln,
                             bias=eps_sb[:, :])
        nc.scalar.activation(out=inv_sb[:, :], in_=var_sb[:, :], func=AF.Exp,
                             scale=-0.5)
        for t in range(nt):
            sl = slice(t * tsz, (t + 1) * tsz)
            nc.gpsimd.tensor_sub(xn[:, sl], x2t[:, sl], mean_sb[:, sl])
            nc.gpsimd.tensor_mul(xn[:, sl], xn[:, sl], inv_sb[:, sl])
            nc.gpsimd.tensor_copy(xnbf[:, sl], xn[:, sl])
        mds = []
        for t in range(nt):
            sl = slice(t * tsz, (t + 1) * tsz)
            dps = psz1.tile([C, 512], F32, tag="z1", name="dps")
            nc.tensor.matmul(dps[:, 0:tsz], diw, xnbf[:, sl],
                             start=True, stop=True)
            md = mdpool.tile([C, tsz], BF, tag="md", name="md")
            nc.vector._custom_dve(GELU2_OP, out=md[:, :], in0=dps[:, 0:tsz],
                                  s0=bcol(12, l), s1=bcol(9, l), imm2=C2F)
            mds.append(md)
        for t in range(nt):
            sl = slice(t * tsz, (t + 1) * tsz)
            dps2 = psz2.tile([C, 512], F32, tag="z2", name="dps2")
            nc.tensor.matmul(dps2[:, 0:tsz], dow, mds[t][:, :],
                             start=True, stop=True)
            nc.vector.affine_then_add(x2t[:, sl], xn[:, sl], dps2[:, 0:tsz],
                                      scale=lncol(0, l), bias=lncol(3, l))
            stats_of(sl, x2t[:, sl])
        nc.scalar.activation(out=var_sb[:, :], in_=var_sb[:, :], func=AF.Ln,
                             bias=eps_sb[:, :])
        nc.scalar.activation(out=inv_sb[:, :], in_=var_sb[:, :], func=AF.Exp,
                             scale=-0.5)
        for t in range(nt):
            sl = slice(t * tsz, (t + 1) * tsz)
            nc.gpsimd.tensor_sub(xn[:, sl], x2t[:, sl], mean_sb[:, sl])
            nc.gpsimd.tensor_mul(xn[:, sl], xn[:, sl], inv_sb[:, sl])
            dst = h_bufs[l % 2] if l < L - 1 else mean_sb
            nc.vector._custom_dve(AFFINE_MUL_REDUCE, out=dst[:, sl],
                                  in0=xn[:, sl], in1=mask_rep[:, sl],
                                  s0=lncol(6, l), s1=lncol(9, l))
            if l < L - 1:
                nc.gpsimd.tensor_copy(hsh[:, sl], dst[:, sl])
                emit_S(l + 1, t, hsh)
            else:
                nc.sync.dma_start(out=io["out_hT"][:, sl], in_=mean_sb[:, sl])


def build_nc(nper=NPER, tsz=T):
    nc = bacc.Bacc("TRN2", target_bir_lowering=False, debug=False,
                   enable_asserts=False)
    io = {
        "efT": nc.dram_tensor("efT", [C, K, nper], BF, kind="ExternalInput").ap(),
        "nfT": nc.dram_tensor("nfT", [C, nper], BF, kind="ExternalInput").ap(),
        "maskT": nc.dram_tensor("maskT", [1, nper], BF, kind="ExternalInput").ap(),
        "bvec": nc.dram_tensor("bvec", [C, 15], F32, kind="ExternalInput").ap(),
        "lnvec": nc.dram_tensor("lnvec", [C, 12], F32, kind="ExternalInput").ap(),
        "out_hT": nc.dram_tensor("out_hT", [C, nper], F32, kind="ExternalOutput").ap(),
    }
    for nm in ("w1aT", "w1bT", "w1eT", "w2T", "w3sT", "diwT", "dowT"):
        io[nm] = nc.dram_tensor(nm, [L, C, H], BF, kind="ExternalInput").ap()
    with tile.TileContext(nc) as tc:
        with ExitStack() as ctx:
            _emit(ctx, tc, io, nper, tsz)
    nc.compile()
    return nc


def host_prep(inputs, nper=NPER, ncores=NCORES):
    """Shard + lay out inputs for the device. Returns list of per-core in_maps."""
    bf = ml_dtypes.bfloat16
    nf = np.asarray(inputs["node_features"], np.float32)
    ef = np.asarray(inputs["edge_features"], np.float32)
    mask = np.asarray(inputs["mask"], np.float32)
    w1 = np.asarray(inputs["w1"], np.float32)
    w2 = np.asarray(inputs["w2"], np.float32)
    w3 = np.asarray(inputs["w3"], np.float32)
    di_w = np.asarray(inputs["di_w"], np.float32)
    do_w = np.asarray(inputs["do_w"], np.float32)
    b1 = np.asarray(inputs["b1"], np.float32)
    b2 = np.asarray(inputs["b2"], np.float32)
    b3 = np.asarray(inputs["b3"], np.float32)
    di_b = np.asarray(inputs["di_b"], np.float32)
    do_b = np.asarray(inputs["do_b"], np.float32)
    n1_s = np.asarray(inputs["n1_s"], np.float32)
    n1_b = np.asarray(inputs["n1_b"], np.float32)
    n2_s = np.asarray(inputs["n2_s"], np.float32)
    n2_b = np.asarray(inputs["n2_b"], np.float32)

    def tr(w):  # (L, A, B) -> (L, B, A) contiguous bf16
        return np.ascontiguousarray(w.transpose(0, 2, 1)).astype(bf)

    def trf(w):  # (L, A, B) -> (L, B, A) contiguous f32
        return np.ascontiguousarray(w.transpose(0, 2, 1)).astype(np.float32)

    # di_w with gamma1 folded into its columns, times LAM
    diw_g = di_w * n1_s[:, None, :]
    di_b_eff = di_b + np.einsum('lhc,lc->lh', di_w, n1_b)

    shared = {
        "w1aT": tr(LAM * w1[:, :, 0:C]),
        "w1bT": tr(LAM * w1[:, :, C:2 * C]),
        "w1eT": tr(LAM * w1[:, :, 3 * C:4 * C]),
        "w2T": tr(MU * w2),
        "w3sT": tr(w3 / SCALE),
        "diwT": tr(LAM * diw_g),
        "dowT": tr(MU * do_w),
    }
    bvec = np.zeros((C, 15), np.float32)
    lnvec = np.zeros((C, 12), np.float32)
    for l in range(L):
        bvec[:, 0 + l] = LAM * b1[l] + C1F          # S' copy bias
        bvec[:, 3 + l] = b2[l]                      # ACT gelu2 bias
        bvec[:, 6 + l] = b3[l] * K / SCALE          # x2t copy bias
        bvec[:H, 9 + l] = LAM * di_b_eff[l] + C1F   # md gelu C1
        bvec[:H, 12 + l] = LAM * di_b_eff[l]        # md gelu C0
        lnvec[:, 0 + l] = n1_s[l]                   # affine_then_add scale
        lnvec[:, 3 + l] = n1_b[l] + do_b[l]         # affine_then_add bias
        lnvec[:, 6 + l] = n2_s[l]                   # h-assemble scale
        lnvec[:, 9 + l] = n2_b[l]                   # h-assemble bias
    shared["bvec"] = bvec
    shared["lnvec"] = lnvec

    in_maps = []
    for c in range(ncores):
        sl = slice(c * nper, (c + 1) * nper)
        efc = ef[sl].astype(bf)                              # (nper, K, C)
        in_maps.append(dict(
            efT=np.ascontiguousarray(efc.transpose(2, 1, 0)),  # (C, K, nper)
            nfT=np.ascontiguousarray(nf[sl].T).astype(bf),
            maskT=mask[sl].reshape(1, nper).astype(bf),
            **shared,
        ))
    return in_maps


_NC_CACHE = {}


def kernel(**inputs):
    in_maps = host_prep(inputs)
    if "nc" not in _NC_CACHE:
        _NC_CACHE["nc"] = build_nc()
    nc = _NC_CACHE["nc"]
    res = run_bass_kernel_spmd(nc, in_maps, core_ids=list(range(NCORES)))
    out = np.concatenate([np.asarray(res.results[c]["out_hT"]).T
                          for c in range(NCORES)], axis=0)
    return np.ascontiguousarray(out.astype(np.float32))
